# revision 10
# baseline (speedup 1.0000x reference)
"""Trainium2 Bass kernel: cross-attention (K/V/Q proj + masked softmax +
query-mask + AV + residual + LayerNorm) for B=8, Sq=Sk=1024, H=512, 4 heads.

Sharding: data-parallel over batch — core b handles batch element b.
Weights / LayerNorm params are replicated (LN affine applied on host since
gamma/beta are identity in this problem; applied numerically if not).

Per-core dataflow (all matmuls on PE in fp32r = TF32-like, 1 cyc/row at
N=512; elementwise spread over ACT/DVE/GPSIMD):
  phase 1:  qT = WqT.T-chunks @ decT    -> [128d, 4h x 1024s]  (transposed q)
            kT = WkT.T-chunks @ memT    -> [128d, 4h x 1024s]
            v  = memT.T-chunks @ WvT    -> 8 x [128s, 512o]    (natural v)
            mask u8 -> bf16 {0,1} tiles (resident)
  phase 2:  per (head h, q-tile qt):
            scores psum[q128, k1024] = qT-slice.T @ kT  (2 matmuls, N=512)
              += FILL*I_bf16 @ m_bf16  (additive mask via PE accumulation)
            mexp = exp(SCALE * psum) on ACT, accum_out = row sums
            rs = (1/sums) * query_mask   (DVE, [128,1])
            attn = mexp * rs             (GPSIMD tensor_scalar)
            DMA attn -> attn_out[h, qt];  PE-transpose attn -> attnT (bf16)
            per head: AV outT[d, q] = sum_k v-chunk.T @ attnT  (N=512)
  phase 3:  per qt: PE-transpose outT -> out[q, 512]; x = out + dec;
            LayerNorm via bn_stats/bn_aggr + sqrt + reciprocal; DMA result.

softmax skips the max-subtraction: scores/sqrt(128) is bounded (~|3|) so
exp never overflows, and masked entries underflow to exactly 0 through
the additive -2^30 fill.  Fully-masked rows (prob ~2^-1024) would NaN.
"""

import math
import os

import numpy as np

import concourse.bass as bass
import concourse.tile as tile
from concourse import bacc
from concourse import masks, mybir
from concourse.bass_utils import run_bass_kernel_spmd

B, SQ, SK, H, NH = 8, 1024, 1024, 512, 4
D = H // NH  # 128
P = 128
N_CORES = 8
LN_EPS = 1e-5
FILL = -float(2**30)
SCALE = 1.0 / math.sqrt(D)

F32 = mybir.dt.float32
F32R = mybir.dt.float32r
BF16 = mybir.dt.bfloat16
U8 = mybir.dt.uint8

NQT = SQ // P  # 8 q tiles
NKC = SK // P  # 8 k chunks
NIC = H // P  # 4 input-channel chunks
NSC = SK // P  # 8 s chunks (for v)


def _emit(tc, io):
    nc = tc.nc
    memT_d, decT_d, dec_d = io["memT"], io["decT"], io["dec"]
    msk_d, qm_d = io["msk"], io["qm"]
    wqT_d, wkT_d, wvT_d = io["wqT"], io["wkT"], io["wvT"]
    attn_d, res_d = io["attn_out"], io["result"]

    with (
        tc.tile_pool(name="persist", bufs=1) as persist,
        tc.tile_pool(name="small", bufs=6) as small,
        tc.tile_pool(name="work", bufs=3) as work,
        tc.tile_pool(name="pscores", bufs=2, space="PSUM") as pscores,
        tc.tile_pool(name="ptr", bufs=2, space="PSUM") as ptr,
        tc.tile_pool(name="pacc", bufs=2, space="PSUM") as pacc,
    ):
        # ---- constants ----
        ident = persist.tile([P, P], F32, tag="ident")
        masks.make_identity(nc, ident)
        
        filli = persist.tile([P, P], BF16, tag="filli")
        nc.gpsimd.memset(filli, 0.0)
        nc.gpsimd.affine_select(
            out=filli,
            in_=filli,
            compare_op=mybir.AluOpType.not_equal,
            fill=FILL,
            base=0,
            pattern=[[-1, P]],
            channel_multiplier=1,
        )
        eps_t = persist.tile([P, 1], F32, tag="eps")
        nc.vector.memset(eps_t, LN_EPS)
        qm_t = []
        for t in range(NQT):
            q = persist.tile([P, 1], F32, tag=f"qm{t}", name=f"qm{t}")
            nc.sync.dma_start(q, qm_d[t * P : (t + 1) * P, :])
            qm_t.append(q)

        # ---- persistent activations ----
        qT = persist.tile([P, NH * SQ], F32R, tag="qT")  # [d, h*1024 + s]
        kT = persist.tile([P, NH * SK], F32R, tag="kT")
        v_sc = [persist.tile([P, H], BF16, tag=f"v{sc}", name=f"v{sc}") for sc in range(NSC)]

        # ---- phase 1: weights + inputs + projections ----
        with tc.tile_pool(name="p1", bufs=1) as p1:
            def load_rows(dram, name, width, eng):
                tiles = []
                for ic in range(NIC):
                    t = p1.tile([P, width], F32R, tag=f"{name}{ic}", name=f"{name}{ic}")
                    eng.dma_start(t, dram[ic * P : (ic + 1) * P, :])
                    tiles.append(t)
                return tiles

            # q-projection inputs first so PE starts earliest; split across
            # the two HWDGE rings (SP + ACT) to halve issue latency
            wq = load_rows(wqT_d, "wq", H, nc.sync)
            decT_t = load_rows(decT_d, "decT", SQ, nc.scalar)
            wk = load_rows(wkT_d, "wk", H, nc.sync)
            memT_t = load_rows(memT_d, "memT", SK, nc.scalar)
            wv = load_rows(wvT_d, "wv", H, nc.sync)

            # qT / kT projections: psum[d128, s512] = sum_ic WT[ic].T @ xT[ic]
            # both s-halves accumulate together so each weight LDWEIGHTS is
            # shared by two back-to-back matmuls
            def proj_T(w, x_t, dst, base):
                p0 = pacc.tile([P, 512], F32, tag="acc", name="p0")
                p1 = pacc.tile([P, 512], F32, tag="acc", name="p1")
                for ic in range(NIC):
                    w_sl = w[ic][:, h * P : (h + 1) * P]
                    nc.tensor.matmul(p0, w_sl, x_t[ic][:, 0:512], start=(ic == 0), stop=(ic == NIC - 1))
                    nc.tensor.matmul(p1, w_sl, x_t[ic][:, 512:1024], start=(ic == 0), stop=(ic == NIC - 1))
                nc.vector.tensor_copy(dst[:, base : base + 512], p0)
                nc.vector.tensor_copy(dst[:, base + 512 : base + 1024], p1)

            for h in range(NH):
                proj_T(wq, decT_t, qT, h * SQ)
                proj_T(wk, memT_t, kT, h * SK)

            # v: psum[s128, o512] = sum_ic memT[ic]-slice.T @ WvT[ic]
            for sc in range(NSC):
                pv = pacc.tile([P, 512], F32, tag="acc")
                for ic in range(NIC):
                    nc.tensor.matmul(
                        pv,
                        memT_t[ic][:, sc * P : (sc + 1) * P],
                        wv[ic],
                        start=(ic == 0),
                        stop=(ic == NIC - 1),
                    )
                nc.scalar.copy(v_sc[sc], pv)  # ACT is idle in phase 1; cast f32 -> bf16

        # ---- phase 2+3: attention, AV, residual + LayerNorm ----
        # Software-pipelined emission: PE runs iteration i-1's transposes/AV
        # while iteration i's softmax chain (ACT exp -> DVE recip/scale) runs,
        # so the PE never stalls on the cross-engine chain. The normalize is a
        # single two-scalar DVE op: attn = (mexp * (1/sums)) * query_mask.
        def front(qt, h, m_bf):
            ps = pscores.tile([P, SK], F32, tag="scores", name="ps")
            q_sl = qT[:, h * SQ + qt * P : h * SQ + (qt + 1) * P]
            nc.tensor.matmul(ps[:, 0:512], q_sl, kT[:, h * SK : h * SK + 512], start=True, stop=False)
            nc.tensor.matmul(ps[:, 512:1024], q_sl, kT[:, h * SK + 512 : h * SK + 1024], start=True, stop=False)
            nc.tensor.matmul(ps[:, 0:512], filli, m_bf[:, 0:512], start=False, stop=True)
            nc.tensor.matmul(ps[:, 512:1024], filli, m_bf[:, 512:1024], start=False, stop=True)
            mexp = work.tile([P, SK], F32, tag="mexp", name="mexp")
            sums = small.tile([P, 1], F32, tag="sums", name="sums")
            nc.scalar.activation(
                mexp, ps, mybir.ActivationFunctionType.Exp, scale=SCALE, accum_out=sums
            )
            rcp = small.tile([P, 1], F32, tag="rcp", name="rcp")
            nc.vector.reciprocal(rcp, sums)
            attn = work.tile([P, SK], F32, tag="attn", name="attn")
            nc.vector.tensor_scalar(
                attn, mexp, rcp, qm_t[qt],
                op0=mybir.AluOpType.mult, op1=mybir.AluOpType.mult,
            )
            nc.sync.dma_start(attn_d[h, qt * P : (qt + 1) * P, :], attn)
            return attn

        def back(qt, h, attn, pav):
            attnT = work.tile([P, NKC, P], BF16, tag="attnT", bufs=3, name="attnT")
            for g in range(2):
                pt = ptr.tile([P, 512], F32, tag="tr", name="pt")
                for c in range(4):
                    kc = g * 4 + c
                    nc.tensor.transpose(
                        pt[:, c * P : (c + 1) * P],
                        attn[:, kc * P : (kc + 1) * P],
                        ident,
                    )
                dst = attnT[:, g * 4 : (g + 1) * 4, :]
                src_ap = pt.rearrange("p (c q) -> p c q", c=4)
                if g == 0:
                    nc.scalar.copy(dst, src_ap)
                else:
                    nc.vector.tensor_copy(dst, src_ap)
            for kc in range(NKC):
                nc.tensor.matmul(
                    pav[:, h * P : (h + 1) * P],
                    attnT[:, kc, :],
                    v_sc[kc][:, h * P : (h + 1) * P],
                    start=(kc == 0),
                    stop=(kc == NKC - 1),
                )

        def ln_stats(qt, pav):
            dect = work.tile([P, H], F32, tag="dect", bufs=2, name="dect")
            nc.sync.dma_start(dect, dec_d[qt * P : (qt + 1) * P, :])
            x = persist.tile([P, H], F32, tag=f"x{qt}", name=f"x{qt}")
            nc.vector.tensor_tensor(x, pav, dect, mybir.AluOpType.add)
            stats = small.tile([P, nc.vector.BN_STATS_DIM], F32, tag="stats", name="stats")
            nc.vector.bn_stats(stats, x)
            mv = persist.tile([P, nc.vector.BN_AGGR_DIM], F32, tag=f"mv{qt}", name=f"mv{qt}")
            nc.vector.bn_aggr(mv, stats)
            x_t.append(x)
            mv_t.append(mv)

        x_t, mv_t = [], []
        pavs = {}
        m_bf = None
        prev = None
        for qt in range(NQT):
            mu8 = work.tile([P, SK], U8, tag="mu8", bufs=2, name="mu8")
            nc.gpsimd.dma_start(mu8, msk_d[qt * P : (qt + 1) * P, :])
            m_bf = work.tile([P, SK], BF16, tag="mbf", bufs=2, name="mbf")
            nc.gpsimd.tensor_copy(m_bf, mu8)
            pavs[qt] = pacc.tile([P, H], F32, tag="acc", name="pav")
            for h in range(NH):
                attn = front(qt, h, m_bf)
                if prev is not None:
                    pqt, ph, pattn = prev
                    back(pqt, ph, pattn, pavs[pqt])
                    if ph == NH - 1:
                        ln_stats(pqt, pavs[pqt])
                prev = (qt, h, attn)
        pqt, ph, pattn = prev
        back(pqt, ph, pattn, pavs[pqt])
        ln_stats(pqt, pavs[pqt])

        # ---- LayerNorm tail ----
        for qt in range(NQT):
            x, mv = x_t[qt], mv_t[qt]
            std = small.tile([P, 1], F32, tag="std", name="std")
            nc.scalar.activation(
                std, mv[:, 1:2], mybir.ActivationFunctionType.Sqrt, bias=eps_t, scale=1.0
            )
            rstd = small.tile([P, 1], F32, tag="rstd", name="rstd")
            nc.vector.reciprocal(rstd, std)
            res = work.tile([P, H], F32, tag="res", bufs=2, name="res")
            nc.vector.tensor_scalar(
                res,
                x,
                mv[:, 0:1],
                rstd,
                op0=mybir.AluOpType.subtract,
                op1=mybir.AluOpType.mult,
            )
            nc.gpsimd.dma_start(res_d[qt * P : (qt + 1) * P, :], res)


def build_nc():
    nc = bacc.Bacc("TRN2", target_bir_lowering=False, debug=False)
    io = {
        "memT": nc.dram_tensor("memT", [H, SK], F32R, kind="ExternalInput").ap(),
        "decT": nc.dram_tensor("decT", [H, SQ], F32R, kind="ExternalInput").ap(),
        "dec": nc.dram_tensor("dec", [SQ, H], F32, kind="ExternalInput").ap(),
        "msk": nc.dram_tensor("msk", [SQ, SK], U8, kind="ExternalInput").ap(),
        "qm": nc.dram_tensor("qm", [SQ, 1], F32, kind="ExternalInput").ap(),
        "wqT": nc.dram_tensor("wqT", [H, H], F32R, kind="ExternalInput").ap(),
        "wkT": nc.dram_tensor("wkT", [H, H], F32R, kind="ExternalInput").ap(),
        "wvT": nc.dram_tensor("wvT", [H, H], F32R, kind="ExternalInput").ap(),
        "attn_out": nc.dram_tensor("attn_out", [NH, SQ, SK], F32, kind="ExternalOutput").ap(),
        "result": nc.dram_tensor("result", [SQ, H], F32, kind="ExternalOutput").ap(),
    }
    with tile.TileContext(nc) as tc:
        _emit(tc, io)
    nc.compile()
    return nc


_NC = None
LAST_EXEC_NS = None


def _round_fp32r(a):
    """Round fp32 -> the fp32r (TF32-like, 11-bit mantissa) representable set."""
    a = np.ascontiguousarray(a, np.float32)
    try:
        from neuron_dtypes._impl import fp32r as _impl

        bits = a.view(np.uint32).ravel()
        r = _impl.cast_fp32_to_fp32r(bits.size, bits.copy())
        return np.asarray(r, dtype=np.uint32).reshape(a.shape).view(np.float32)
    except Exception:
        bits = a.view(np.uint32)
        return ((bits + 0x800) & np.uint32(0xFFFFF000)).view(np.float32)


def make_in_maps(memory, decoder_input, mask, query_mask, Wk, Wv, Wq):
    wqT = _round_fp32r(np.asarray(Wq, np.float32).T)
    wkT = _round_fp32r(np.asarray(Wk, np.float32).T)
    wvT = _round_fp32r(np.asarray(Wv, np.float32).T)
    in_maps = []
    for b in range(B):
        in_maps.append(
            {
                "memT": _round_fp32r(np.asarray(memory[b], np.float32).T),
                "decT": _round_fp32r(np.asarray(decoder_input[b], np.float32).T),
                "dec": np.ascontiguousarray(np.asarray(decoder_input[b], np.float32)),
                "msk": np.ascontiguousarray(np.asarray(mask[b])).view(np.uint8),
                "qm": np.ascontiguousarray(
                    np.asarray(query_mask[b], np.float32).reshape(SQ, 1)
                ),
                "wqT": wqT,
                "wkT": wkT,
                "wvT": wvT,
            }
        )
    return in_maps


def kernel(memory, decoder_input, mask, query_mask, Wk, Wv, Wq, gamma, beta):
    global _NC, LAST_EXEC_NS
    if _NC is None:
        _NC = build_nc()
    in_maps = make_in_maps(memory, decoder_input, mask, query_mask, Wk, Wv, Wq)
    trace = os.environ.get("BASS_ATTN_TRACE", "0") == "1"
    out = run_bass_kernel_spmd(_NC, in_maps, core_ids=list(range(N_CORES)), trace=trace)
    LAST_EXEC_NS = out.exec_time_ns
    result = np.stack([out.results[b]["result"] for b in range(B)])  # [B, SQ, H]
    attn = np.stack([out.results[b]["attn_out"] for b in range(B)], axis=1).reshape(
        NH * B, SQ, SK
    )
    gamma = np.asarray(gamma, np.float32)
    beta = np.asarray(beta, np.float32)
    if not (np.all(gamma == 1.0) and np.all(beta == 0.0)):
        # device computes (x - mu) * rstd; LN affine is exact to apply here
        result = result * gamma[None, None, :] + beta[None, None, :]
    return result.astype(np.float32), attn.astype(np.float32)


# revision 11
# speedup vs baseline: 1.0499x; 1.0499x over previous
"""Trainium2 Bass kernel: cross-attention (K/V/Q proj + masked softmax +
query-mask + AV + residual + LayerNorm) for B=8, Sq=Sk=1024, H=512, 4 heads.

Sharding: data-parallel over batch — core b handles batch element b.
Weights / LayerNorm params are replicated (LN affine applied on host since
gamma/beta are identity in this problem; applied numerically if not).

Per-core dataflow (all matmuls on PE in fp32r = TF32-like, 1 cyc/row at
N=512; elementwise spread over ACT/DVE/GPSIMD):
  phase 1:  qT = WqT.T-chunks @ decT    -> [128d, 4h x 1024s]  (transposed q)
            kT = WkT.T-chunks @ memT    -> [128d, 4h x 1024s]
            v  = memT.T-chunks @ WvT    -> 8 x [128s, 512o]    (natural v)
            mask u8 -> bf16 {0,1} tiles (resident)
  phase 2:  per (head h, q-tile qt):
            scores psum[q128, k1024] = qT-slice.T @ kT  (2 matmuls, N=512)
              += FILL*I_bf16 @ m_bf16  (additive mask via PE accumulation)
            mexp = exp(SCALE * psum) on ACT, accum_out = row sums
            rs = (1/sums) * query_mask   (DVE, [128,1])
            attn = mexp * rs             (GPSIMD tensor_scalar)
            DMA attn -> attn_out[h, qt];  PE-transpose attn -> attnT (bf16)
            per head: AV outT[d, q] = sum_k v-chunk.T @ attnT  (N=512)
  phase 3:  per qt: PE-transpose outT -> out[q, 512]; x = out + dec;
            LayerNorm via bn_stats/bn_aggr + sqrt + reciprocal; DMA result.

softmax skips the max-subtraction: scores/sqrt(128) is bounded (~|3|) so
exp never overflows, and masked entries underflow to exactly 0 through
the additive -2^30 fill.  Fully-masked rows (prob ~2^-1024) would NaN.
"""

import math
import os

import numpy as np

import concourse.bass as bass
import concourse.tile as tile
from concourse import bacc
from concourse import masks, mybir
from concourse.bass_utils import run_bass_kernel_spmd

B, SQ, SK, H, NH = 8, 1024, 1024, 512, 4
D = H // NH  # 128
P = 128
N_CORES = 8
LN_EPS = 1e-5
FILL = -float(2**30)
SCALE = 1.0 / math.sqrt(D)

F32 = mybir.dt.float32
F32R = mybir.dt.float32r
BF16 = mybir.dt.bfloat16
U8 = mybir.dt.uint8

NQT = SQ // P  # 8 q tiles
NKC = SK // P  # 8 k chunks
NIC = H // P  # 4 input-channel chunks
NSC = SK // P  # 8 s chunks (for v)


def _emit(tc, io):
    nc = tc.nc
    memT_d, decT_d, dec_d = io["memT"], io["decT"], io["dec"]
    msk_d, qm_d = io["msk"], io["qm"]
    wqT_d, wkT_d, wvT_d = io["wqT"], io["wkT"], io["wvT"]
    attn_d, res_d = io["attn_out"], io["result"]

    with (
        tc.tile_pool(name="persist", bufs=1) as persist,
        tc.tile_pool(name="small", bufs=6) as small,
        tc.tile_pool(name="work", bufs=3) as work,
        tc.tile_pool(name="pscores", bufs=2, space="PSUM") as pscores,
        tc.tile_pool(name="ptr", bufs=2, space="PSUM") as ptr,
        tc.tile_pool(name="pacc", bufs=2, space="PSUM") as pacc,
    ):
        # ---- constants ----
        ident = persist.tile([P, P], F32, tag="ident")
        masks.make_identity(nc, ident)
        
        filli = persist.tile([P, P], BF16, tag="filli")
        nc.gpsimd.memset(filli, 0.0)
        nc.gpsimd.affine_select(
            out=filli,
            in_=filli,
            compare_op=mybir.AluOpType.not_equal,
            fill=FILL,
            base=0,
            pattern=[[-1, P]],
            channel_multiplier=1,
        )
        eps_t = persist.tile([P, 1], F32, tag="eps")
        nc.vector.memset(eps_t, LN_EPS)
        qm_t = []
        for t in range(NQT):
            q = persist.tile([P, 1], F32, tag=f"qm{t}", name=f"qm{t}")
            nc.sync.dma_start(q, qm_d[t * P : (t + 1) * P, :])
            qm_t.append(q)

        # ---- persistent activations ----
        qT = persist.tile([P, NH * SQ], F32R, tag="qT")  # [d, h*1024 + s]
        kT = persist.tile([P, NH * SK], F32R, tag="kT")
        v_sc = [persist.tile([P, H], BF16, tag=f"v{sc}", name=f"v{sc}") for sc in range(NSC)]

        # ---- phase 1: weights + inputs + projections ----
        with tc.tile_pool(name="p1", bufs=1) as p1:
            def load_rows(dram, name, width, eng):
                tiles = []
                for ic in range(NIC):
                    t = p1.tile([P, width], F32R, tag=f"{name}{ic}", name=f"{name}{ic}")
                    eng.dma_start(t, dram[ic * P : (ic + 1) * P, :])
                    tiles.append(t)
                return tiles

            # q-projection inputs first so PE starts earliest; split across
            # the two HWDGE rings (SP + ACT) to halve issue latency
            wq = load_rows(wqT_d, "wq", H, nc.sync)
            decT_t = load_rows(decT_d, "decT", SQ, nc.scalar)
            wk = load_rows(wkT_d, "wk", H, nc.sync)
            memT_t = load_rows(memT_d, "memT", SK, nc.scalar)
            wv = load_rows(wvT_d, "wv", H, nc.sync)

            # qT / kT projections: psum[d128, s512] = sum_ic WT[ic].T @ xT[ic]
            # both s-halves accumulate together so each weight LDWEIGHTS is
            # shared by two back-to-back matmuls
            def proj_T(w, x_t, dst, base):
                p0 = pacc.tile([P, 512], F32, tag="acc", name="p0")
                p1 = pacc.tile([P, 512], F32, tag="acc", name="p1")
                for ic in range(NIC):
                    w_sl = w[ic][:, h * P : (h + 1) * P]
                    nc.tensor.matmul(p0, w_sl, x_t[ic][:, 0:512], start=(ic == 0), stop=(ic == NIC - 1))
                    nc.tensor.matmul(p1, w_sl, x_t[ic][:, 512:1024], start=(ic == 0), stop=(ic == NIC - 1))
                nc.vector.tensor_copy(dst[:, base : base + 512], p0)
                nc.vector.tensor_copy(dst[:, base + 512 : base + 1024], p1)

            for h in range(NH):
                proj_T(wq, decT_t, qT, h * SQ)
                proj_T(wk, memT_t, kT, h * SK)

            # v: psum[s128, o512] = sum_ic memT[ic]-slice.T @ WvT[ic]
            for sc in range(NSC):
                pv = pacc.tile([P, 512], F32, tag="acc")
                for ic in range(NIC):
                    nc.tensor.matmul(
                        pv,
                        memT_t[ic][:, sc * P : (sc + 1) * P],
                        wv[ic],
                        start=(ic == 0),
                        stop=(ic == NIC - 1),
                    )
                nc.scalar.copy(v_sc[sc], pv)  # ACT is idle in phase 1; cast f32 -> bf16

        # ---- phase 2+3: attention, AV, residual + LayerNorm ----
        # Software-pipelined emission: PE runs iteration i-1's transposes/AV
        # while iteration i's softmax chain (ACT exp -> DVE recip/scale) runs,
        # so the PE never stalls on the cross-engine chain. The normalize is a
        # single two-scalar DVE op: attn = (mexp * (1/sums)) * query_mask.
        def front(qt, h, m_bf):
            ps = pscores.tile([P, SK], F32, tag="scores", name="ps")
            q_sl = qT[:, h * SQ + qt * P : h * SQ + (qt + 1) * P]
            nc.tensor.matmul(ps[:, 0:512], q_sl, kT[:, h * SK : h * SK + 512], start=True, stop=False)
            nc.tensor.matmul(ps[:, 512:1024], q_sl, kT[:, h * SK + 512 : h * SK + 1024], start=True, stop=False)
            nc.tensor.matmul(ps[:, 0:512], filli, m_bf[:, 0:512], start=False, stop=True)
            nc.tensor.matmul(ps[:, 512:1024], filli, m_bf[:, 512:1024], start=False, stop=True)
            mexp = work.tile([P, SK], F32, tag="mexp", name="mexp")
            sums = small.tile([P, 1], F32, tag="sums", name="sums")
            nc.scalar.activation(
                mexp, ps, mybir.ActivationFunctionType.Exp, scale=SCALE, accum_out=sums
            )
            rcp = small.tile([P, 1], F32, tag="rcp", name="rcp")
            nc.vector.reciprocal(rcp, sums)
            attn = work.tile([P, SK], F32, tag="attn", bufs=5, name="attn")
            nc.vector.tensor_scalar(
                attn, mexp, rcp, qm_t[qt],
                op0=mybir.AluOpType.mult, op1=mybir.AluOpType.mult,
            )
            nc.sync.dma_start(attn_d[h, qt * P : (qt + 1) * P, :], attn)
            return attn

        def back(qt, h, attn, pav):
            attnT = work.tile([P, NKC, P], BF16, tag="attnT", bufs=3, name="attnT")
            for g in range(2):
                pt = ptr.tile([P, 512], F32, tag="tr", name="pt")
                for c in range(4):
                    kc = g * 4 + c
                    nc.tensor.transpose(
                        pt[:, c * P : (c + 1) * P],
                        attn[:, kc * P : (kc + 1) * P],
                        ident,
                    )
                dst = attnT[:, g * 4 : (g + 1) * 4, :]
                src_ap = pt.rearrange("p (c q) -> p c q", c=4)
                if g == 0:
                    nc.scalar.copy(dst, src_ap)
                else:
                    nc.vector.tensor_copy(dst, src_ap)
            for kc in range(NKC):
                nc.tensor.matmul(
                    pav[:, h * P : (h + 1) * P],
                    attnT[:, kc, :],
                    v_sc[kc][:, h * P : (h + 1) * P],
                    start=(kc == 0),
                    stop=(kc == NKC - 1),
                )

        def ln_stats(qt, pav):
            dect = work.tile([P, H], F32, tag="dect", bufs=2, name="dect")
            nc.sync.dma_start(dect, dec_d[qt * P : (qt + 1) * P, :])
            x = persist.tile([P, H], F32, tag=f"x{qt}", name=f"x{qt}")
            nc.vector.tensor_tensor(x, pav, dect, mybir.AluOpType.add)
            stats = small.tile([P, nc.vector.BN_STATS_DIM], F32, tag="stats", name="stats")
            nc.vector.bn_stats(stats, x)
            mv = persist.tile([P, nc.vector.BN_AGGR_DIM], F32, tag=f"mv{qt}", name=f"mv{qt}")
            nc.vector.bn_aggr(mv, stats)
            x_t.append(x)
            mv_t.append(mv)

        def load_mask(qt):
            mu8 = work.tile([P, SK], U8, tag="mu8", bufs=2, name="mu8")
            nc.gpsimd.dma_start(mu8, msk_d[qt * P : (qt + 1) * P, :])
            m_bf = work.tile([P, SK], BF16, tag="mbf", bufs=2, name="mbf")
            nc.gpsimd.tensor_copy(m_bf, mu8)
            return m_bf

        x_t, mv_t = [], []
        pavs = {}
        prev = None
        m_cur = load_mask(0)
        for qt in range(NQT):
            pavs[qt] = pacc.tile([P, H], F32, tag="acc", name="pav")
            for h in range(NH):
                attn = front(qt, h, m_cur)
                if h == 0 and qt + 1 < NQT:
                    m_next = load_mask(qt + 1)  # prefetch a full qt ahead
                if prev is not None:
                    pqt, ph, pattn = prev
                    back(pqt, ph, pattn, pavs[pqt])
                    if ph == NH - 1:
                        ln_stats(pqt, pavs[pqt])
                prev = (qt, h, attn)
            m_cur = m_next
        pqt, ph, pattn = prev
        back(pqt, ph, pattn, pavs[pqt])
        ln_stats(pqt, pavs[pqt])

        # ---- LayerNorm tail ----
        for qt in range(NQT):
            x, mv = x_t[qt], mv_t[qt]
            std = small.tile([P, 1], F32, tag="std", name="std")
            nc.scalar.activation(
                std, mv[:, 1:2], mybir.ActivationFunctionType.Sqrt, bias=eps_t, scale=1.0
            )
            rstd = small.tile([P, 1], F32, tag="rstd", name="rstd")
            nc.vector.reciprocal(rstd, std)
            res = work.tile([P, H], F32, tag="res", bufs=2, name="res")
            nc.vector.tensor_scalar(
                res,
                x,
                mv[:, 0:1],
                rstd,
                op0=mybir.AluOpType.subtract,
                op1=mybir.AluOpType.mult,
            )
            nc.gpsimd.dma_start(res_d[qt * P : (qt + 1) * P, :], res)


def build_nc():
    nc = bacc.Bacc("TRN2", target_bir_lowering=False, debug=False)
    io = {
        "memT": nc.dram_tensor("memT", [H, SK], F32R, kind="ExternalInput").ap(),
        "decT": nc.dram_tensor("decT", [H, SQ], F32R, kind="ExternalInput").ap(),
        "dec": nc.dram_tensor("dec", [SQ, H], F32, kind="ExternalInput").ap(),
        "msk": nc.dram_tensor("msk", [SQ, SK], U8, kind="ExternalInput").ap(),
        "qm": nc.dram_tensor("qm", [SQ, 1], F32, kind="ExternalInput").ap(),
        "wqT": nc.dram_tensor("wqT", [H, H], F32R, kind="ExternalInput").ap(),
        "wkT": nc.dram_tensor("wkT", [H, H], F32R, kind="ExternalInput").ap(),
        "wvT": nc.dram_tensor("wvT", [H, H], F32R, kind="ExternalInput").ap(),
        "attn_out": nc.dram_tensor("attn_out", [NH, SQ, SK], F32, kind="ExternalOutput").ap(),
        "result": nc.dram_tensor("result", [SQ, H], F32, kind="ExternalOutput").ap(),
    }
    with tile.TileContext(nc) as tc:
        _emit(tc, io)
    nc.compile()
    return nc


_NC = None
LAST_EXEC_NS = None


def _round_fp32r(a):
    """Round fp32 -> the fp32r (TF32-like, 11-bit mantissa) representable set."""
    a = np.ascontiguousarray(a, np.float32)
    try:
        from neuron_dtypes._impl import fp32r as _impl

        bits = a.view(np.uint32).ravel()
        r = _impl.cast_fp32_to_fp32r(bits.size, bits.copy())
        return np.asarray(r, dtype=np.uint32).reshape(a.shape).view(np.float32)
    except Exception:
        bits = a.view(np.uint32)
        return ((bits + 0x800) & np.uint32(0xFFFFF000)).view(np.float32)


def make_in_maps(memory, decoder_input, mask, query_mask, Wk, Wv, Wq):
    wqT = _round_fp32r(np.asarray(Wq, np.float32).T)
    wkT = _round_fp32r(np.asarray(Wk, np.float32).T)
    wvT = _round_fp32r(np.asarray(Wv, np.float32).T)
    in_maps = []
    for b in range(B):
        in_maps.append(
            {
                "memT": _round_fp32r(np.asarray(memory[b], np.float32).T),
                "decT": _round_fp32r(np.asarray(decoder_input[b], np.float32).T),
                "dec": np.ascontiguousarray(np.asarray(decoder_input[b], np.float32)),
                "msk": np.ascontiguousarray(np.asarray(mask[b])).view(np.uint8),
                "qm": np.ascontiguousarray(
                    np.asarray(query_mask[b], np.float32).reshape(SQ, 1)
                ),
                "wqT": wqT,
                "wkT": wkT,
                "wvT": wvT,
            }
        )
    return in_maps


def kernel(memory, decoder_input, mask, query_mask, Wk, Wv, Wq, gamma, beta):
    global _NC, LAST_EXEC_NS
    if _NC is None:
        _NC = build_nc()
    in_maps = make_in_maps(memory, decoder_input, mask, query_mask, Wk, Wv, Wq)
    trace = os.environ.get("BASS_ATTN_TRACE", "0") == "1"
    out = run_bass_kernel_spmd(_NC, in_maps, core_ids=list(range(N_CORES)), trace=trace)
    LAST_EXEC_NS = out.exec_time_ns
    result = np.stack([out.results[b]["result"] for b in range(B)])  # [B, SQ, H]
    attn = np.stack([out.results[b]["attn_out"] for b in range(B)], axis=1).reshape(
        NH * B, SQ, SK
    )
    gamma = np.asarray(gamma, np.float32)
    beta = np.asarray(beta, np.float32)
    if not (np.all(gamma == 1.0) and np.all(beta == 0.0)):
        # device computes (x - mu) * rstd; LN affine is exact to apply here
        result = result * gamma[None, None, :] + beta[None, None, :]
    return result.astype(np.float32), attn.astype(np.float32)
